# revision 26
# baseline (speedup 1.0000x reference)
"""Trainium2 Bass kernel for nn_CausalSelfAttention_74268574482879.

The reference module's attention scores are overwritten by the causal mask
(q/k are discarded), so softmax weights are uniform over positions <= t:
    y = cummean_T(x) @ W_v @ W_p

Host-side algebra (all exact up to fp rounding):
  * W_c = W_v @ W_p is folded into a single 512x512 weight.
  * The 4096 rows of (B*T) are split into 8 chunks of 512 rows, one per
    NeuronCore.  The cross-chunk carry (column-sum of all preceding rows in
    the same batch element) is added into column 0 of the transposed chunk
    on the host, so the device computes a plain local cumsum.
  * x is passed TRANSPOSED (feature-major) so the cumsum runs as a DVE /
    GpSimd ``tensor_tensor_scan`` along the free (time) dim — no PE work.
  * Everything is cast to bf16 on the host (rel-err budget is 2e-2).

Per-core dataflow:
  scan_i : A^T_i[f, t] = cumsum_t(xT_i[f, t])   (DVE/GpSimd, bf16 out)
  M      : psY_j = sum_i A_i[tile j]^T-slice @ W_c rows i   (16 matmuls)
  evict  : ysb_j = psY_j * 1/(t+1)  (per-partition scalar, DVE/ACT/GpSimd)
A few throwaway matmuls on memset data run during the initial DMA fill to
lift the PE HAM clock-gate early.
"""

import numpy as np
import ml_dtypes

import concourse.bass as bass
import concourse.bacc as bacc
import concourse.mybir as mybir
import concourse.tile as tile
from concourse import bass_utils

N_CORES = 8
B, T, C = 2, 2048, 512
CHUNK = 512               # rows of flattened (B*T) per core
P = 128
NT = CHUNK // P           # 4 row-tiles per chunk
NI = C // P               # 4 col-tiles of the 512 feature dim
F32 = mybir.dt.float32
BF16 = mybir.dt.bfloat16
BF16_NP = ml_dtypes.bfloat16
ADD = mybir.AluOpType.add
BYPASS = mybir.AluOpType.bypass

N_WARM = [8]              # warmup matmuls (HAM unthrottle) during DMA fill
TRACE = [False]
LAST_RESULT = [None]
_STATE = {}


def _build_nc(n_warm):
    nc = bacc.Bacc(
        "TRN2", target_bir_lowering=False, debug=False, num_devices=N_CORES
    )

    xt_d = nc.dram_tensor("xt", (C, CHUNK), BF16, kind="ExternalInput")
    wc_d = nc.dram_tensor("wc", (C, C), BF16, kind="ExternalInput")
    sc_d = nc.dram_tensor("sc", (P, NT), F32, kind="ExternalInput")
    y_d = nc.dram_tensor("y", (CHUNK, C), BF16, kind="ExternalOutput")

    xt_ap, wc_ap, sc_ap, y_ap = xt_d.ap(), wc_d.ap(), sc_d.ap(), y_d.ap()

    with tile.TileContext(nc) as tc:
        with (
            tc.tile_pool(name="io", bufs=1) as io,
            tc.tile_pool(name="ps", bufs=1, space="PSUM") as ps_pool,
        ):
            # ---- input DMAs first, spread over three rings so the scan
            # chain's gates (xt0, xt1, xt23) land as early as possible:
            #   sync   : xt0, then y0/y2 later
            #   scalar : xt1, wc123 (descriptor gen done long before the
            #            ACT evictions run, so no sequencer conflict)
            #   gpsimd : wc0, xt23, sc, then y1/y3 later
            # HWDGE rings drain FIFO per ring, so issue order on one ring
            # doubles as transfer priority.  sync carries the scan-chain
            # gates in need-order; scalar carries xt1 + wc0.
            xt0 = io.tile([P, CHUNK], BF16, name="xt0")
            nc.sync.dma_start(xt0[:], xt_ap[0:P, :])
            xt1 = io.tile([P, CHUNK], BF16, name="xt1")
            nc.scalar.dma_start(xt1[:], xt_ap[P : 2 * P, :])
            xt23 = io.tile([P, 2, CHUNK], BF16, name="xt23")
            nc.sync.dma_start(
                xt23[:], xt_ap[2 * P :, :].rearrange("(i p) t -> p i t", p=P)
            )
            wc0 = io.tile([P, C], BF16, name="wc0")
            nc.scalar.dma_start(wc0[:], wc_ap[0:P, :])
            wc1 = io.tile([P, C], BF16, name="wc1")
            nc.sync.dma_start(wc1[:], wc_ap[P : 2 * P, :])
            wc23 = io.tile([P, 2, C], BF16, name="wc23")
            nc.sync.dma_start(
                wc23[:], wc_ap[2 * P :, :].rearrange("(i p) c -> p i c", p=P)
            )
            xts = [xt0, xt1, xt23[:, 0, :], xt23[:, 1, :]]
            wcs = [wc0, wc1, wc23[:, 0, :], wc23[:, 1, :]]
            scs = io.tile([P, NT], F32, name="scs")
            nc.gpsimd.dma_start(scs[:], sc_ap[:, :])

            # ---- warmup: keep PE busy from t=0 so HAM unthrottles ----
            warm = io.tile([P, C], BF16, name="warm")
            nc.vector.memset(warm[:], 0.0)
            if n_warm:
                wps = ps_pool.tile([P, C], F32, name="wps", tag="warm")
                for _ in range(n_warm):
                    nc.tensor.matmul(
                        wps[:], warm[:, 0:P], warm[:], start=True, stop=True
                    )

            # ---- cumsum scans (DVE-only op), full tiles: the matmul waves
            # chase ~1.3us behind the chain anyway, so sub-tile splitting
            # only adds per-instruction overhead ----
            A_sb = [io.tile([P, CHUNK], BF16, name=f"A{i}") for i in range(NI)]
            H = CHUNK // 2
            for i in range(NI - 1):
                nc.vector.tensor_tensor_scan(
                    A_sb[i][:], xts[i][:], xts[i][:], 0.0, ADD, BYPASS
                )
            # last tile in chained halves: M(3) j0/j1 overlap scan_3b
            nc.vector.tensor_tensor_scan(
                A_sb[3][:, 0:H], xts[3][:, 0:H], xts[3][:, 0:H], 0.0, ADD, BYPASS
            )
            nc.vector.tensor_tensor_scan(
                A_sb[3][:, H:CHUNK], xts[3][:, H:CHUNK], xts[3][:, H:CHUNK],
                A_sb[3][:, H - 1 : H], ADD, BYPASS,
            )

            # ---- stage M: psY_j += A_i[:, tile j]^T @ Wc rows i ----
            psY = [
                ps_pool.tile([P, C], F32, name=f"psY{j}", tag=f"Y{j}")
                for j in range(NT)
            ]
            for i in range(NI):
                for j in range(NT):
                    nc.tensor.matmul(
                        psY[j][:],
                        A_sb[i][:, j * P : (j + 1) * P],
                        wcs[i][:],
                        start=(i == 0),
                        stop=(i == NI - 1),
                    )
            # evictions all issued before any output DMA so no DIRECT2D
            # descriptor-gen lands between two ACT dispatches on the
            # scalar sequencer
            ysbs = []
            for j in range(NT):
                ysb = io.tile([P, C], BF16, name=f"y{j}")
                if j % 2 == 0:
                    nc.scalar.mul(ysb[:], psY[j][:], scs[:, j : j + 1])
                else:
                    nc.vector.tensor_scalar_mul(
                        ysb[:], psY[j][:], scs[:, j : j + 1]
                    )
                ysbs.append(ysb)
            for j in range(NT):
                deng = nc.sync if j % 2 == 0 else nc.scalar
                deng.dma_start(y_ap[j * P : (j + 1) * P, :], ysbs[j][:])

    nc.compile()
    return nc


def _get_nc():
    key = N_WARM[0]
    if key not in _STATE:
        _STATE[key] = _build_nc(key)
    return _STATE[key]


def _prepare_in_maps(x, w_attn, w_proj):
    x = np.asarray(x, dtype=np.float32)
    w_attn = np.asarray(w_attn, dtype=np.float32)
    w_proj = np.asarray(w_proj, dtype=np.float32)
    wc = (w_attn[:, 2 * C : 3 * C] @ w_proj).astype(BF16_NP)

    in_maps = []
    for core in range(N_CORES):
        b, tc_ = divmod(core, T // CHUNK)
        goff = tc_ * CHUNK
        xt = np.array(x[b, goff : goff + CHUNK, :].T, dtype=np.float32)
        if goff:
            # fold the carry into t=0 (cumsum then includes it everywhere)
            xt[:, 0] += x[b, :goff, :].sum(axis=0, dtype=np.float32)
        sc = (
            1.0 / (goff + 1 + np.arange(CHUNK, dtype=np.float32))
        ).reshape(NT, P).T.astype(np.float32)
        in_maps.append(
            {
                "xt": np.ascontiguousarray(xt).astype(BF16_NP),
                "wc": wc,
                "sc": np.ascontiguousarray(sc),
            }
        )
    return in_maps


def kernel(x, w_attn, w_proj):
    nc = _get_nc()
    in_maps = _prepare_in_maps(x, w_attn, w_proj)
    res = bass_utils.run_bass_kernel_spmd(
        nc, in_maps, core_ids=list(range(N_CORES)), trace=TRACE[0]
    )
    LAST_RESULT[0] = res
    y = np.empty((B, T, C), np.float32)
    for core in range(N_CORES):
        b, tc_ = divmod(core, T // CHUNK)
        y[b, tc_ * CHUNK : (tc_ + 1) * CHUNK, :] = res.results[core]["y"].astype(
            np.float32
        )
    return y
